# revision 2
# baseline (speedup 1.0000x reference)
import sys

for _p in ('/opt/trn_rl_repo', '/root/.axon_site'):
    if _p not in sys.path:
        sys.path.insert(0, _p)

import numpy as np

B, H, W = 8, 512, 512
K = 3
NCORES = 8
# padded image: 1 zero row/col before, 2 zero rows/cols after (cols padded
# further so shifted views stay in range and rows stay 4B-aligned)
HP, WP = H + 3, W + 8
NBLK = 4          # row blocks of 128 partitions packed along the free dim
AW = 520          # A tile width (Ipad cols 0..519)
DW = 516          # Dx/Dy/Dxy tile width

# prep ops that run on GpSimd instead of DVE (its SBUF port is free of
# SWDGE duty now that offsets arrive pre-cast, and tensor_tensor never
# takes the shared DVE port pair)
GP_DX = (1, 2)
GP_DY = (0, 1)
GP_DXY = (0, 1)
GP_T_TAPS = (6, 7, 8)   # taps whose t = lx*Dx runs on GpSimd

_compiled = None


def _build():
    import concourse.bacc as bacc
    import concourse.mybir as mybir
    from concourse.tile import TileContext

    f32, f16 = mybir.dt.float32, mybir.dt.float16
    ALU = mybir.AluOpType
    ACTF = mybir.ActivationFunctionType

    nc = bacc.Bacc("TRN2", target_bir_lowering=False, debug=False,
                   num_devices=NCORES)
    ipad = nc.dram_tensor("ipad", [HP, WP], f16, kind="ExternalInput")
    # offsets host-cast to fp16 and pre-packed to the SBUF tile layout:
    # offp[p, k, e, j, c] = offset[2k+e, 128j+p, c]  (e: 0=ly, 1=lx)
    offp = nc.dram_tensor("offp", [128, K * K, 2, NBLK, W], f16,
                          kind="ExternalInput")
    # stack of diag(w_k) matrices used as PE stationary weights
    wdg = nc.dram_tensor("wdg", [128, K * K, 128], f16, kind="ExternalInput")
    out = nc.dram_tensor("out", [H, W], f32, kind="ExternalOutput")

    with TileContext(nc) as tc:
        with (
            tc.tile_pool(name="img", bufs=1) as ip,
            tc.tile_pool(name="lylx", bufs=6) as lp,
            tc.tile_pool(name="tmp", bufs=3) as tp,
            tc.tile_pool(name="cst", bufs=1) as cp,
            tc.tile_pool(name="psum", bufs=1, space="PSUM") as pp,
        ):
            # image row-shifted copies (the two HWDGE rings), earliest first:
            # DVE prep for ky=-1 only needs A[-1], A[0]
            A = {}

            def load_img(dy, eng):
                A[dy] = ip.tile([128, NBLK, AW], f16, tag=f"A{dy}",
                                name=f"A{dy}")
                eng.dma_start(
                    out=A[dy][:],
                    in_=ipad[dy + 1:dy + 513, 0:AW].rearrange(
                        "(j p) c -> p j c", p=128))

            load_img(-1, nc.sync)
            load_img(0, nc.scalar)

            wd = cp.tile([128, K * K, 128], f16, name="wd")
            nc.sync.dma_start(out=wd[:], in_=wdg[:])
            load_img(1, nc.scalar)
            load_img(2, nc.sync)

            # per-tap (ly, lx) pair: one plain HWDGE load each, fp16 in HBM
            lylx = {}

            def load_lylx(k, eng):
                lylx[k] = lp.tile([128, 2, NBLK, W], f16, tag="l",
                                  name=f"l{k}")
                eng.dma_start(out=lylx[k][:], in_=offp[:, k])

            for k in range(K * K):
                load_lylx(k, nc.sync if k % 2 == 0 else nc.scalar)

            psum = pp.tile([128, NBLK, W], f32, name="psum")

            Dx, Dy, Dxy = {}, {}, {}

            def make_dx(dy, eng):
                # Dx = horizontal difference of the padded image
                Dx[dy] = ip.tile([128, NBLK, DW], f16, tag=f"D{dy}",
                                 name=f"D{dy}")
                eng.tensor_tensor(Dx[dy][:], A[dy][:, :, 1:1 + DW],
                                  A[dy][:, :, 0:DW], ALU.subtract)

            def make_dy(j, eng):
                # Dy = vertical difference of the padded image
                Dy[j] = ip.tile([128, NBLK, DW], f16, tag=f"Y{j}",
                                name=f"Y{j}")
                eng.tensor_tensor(Dy[j][:], A[j + 1][:, :, 0:DW],
                                  A[j][:, :, 0:DW], ALU.subtract)

            def make_dxy(j, eng):
                # Dxy = vertical difference of Dx (cross term)
                Dxy[j] = ip.tile([128, NBLK, DW], f16, tag=f"X{j}",
                                 name=f"X{j}")
                eng.tensor_tensor(Dxy[j][:], Dx[j + 1][:],
                                  Dx[j][:], ALU.subtract)

            def eng_dx(dy):
                return nc.gpsimd if dy in GP_DX else nc.vector

            def eng_dy(j):
                return nc.gpsimd if j in GP_DY else nc.vector

            def eng_dxy(j):
                return nc.gpsimd if j in GP_DXY else nc.vector

            def iview(dy, q):
                return A[dy][:, :, q:q + W]

            # per tap: v*w_k = w_k*I0 + w_k*m0 + w_k*u
            #   m0 = lx*Dx[ky]
            #   u  = ly*(Dy[ky] + lx*Dxy[ky])
            for k in range(K * K):
                ky, kx = k // K - 1, k % K - 1
                q = kx + 1
                if kx == -1:
                    if ky not in Dx:
                        make_dx(ky, eng_dx(ky))
                    if ky + 1 not in Dx:
                        make_dx(ky + 1, eng_dx(ky + 1))
                    if ky not in Dy:
                        make_dy(ky, eng_dy(ky))
                    if ky not in Dxy:
                        make_dxy(ky, eng_dxy(ky))
                ll = lylx[k]
                ly, lx = ll[:, 0], ll[:, 1]

                t = tp.tile([128, NBLK, W], f16, tag="t", name="t")
                t2 = tp.tile([128, NBLK, W], f16, tag="t2", name="t2")
                t3 = tp.tile([128, NBLK, W], f16, tag="t3", name="t3")
                teng = nc.gpsimd if k in GP_T_TAPS else nc.vector
                teng.tensor_tensor(t[:], lx[:], Dx[ky][:, :, q:q + W],
                                   ALU.mult)
                nc.vector.tensor_tensor(t3[:], lx[:], Dxy[ky][:, :, q:q + W],
                                        ALU.mult)
                nc.vector.tensor_tensor(t2[:], t3[:], Dy[ky][:, :, q:q + W],
                                        ALU.add)
                nc.vector.tensor_tensor(t2[:], ly[:], t2[:], ALU.mult)

                wk = wd[:, k, :]
                last = k == K * K - 1
                for j in range(NBLK):
                    nc.tensor.matmul(psum[:, j, :], wk, iview(ky, q)[:, j, :],
                                     start=(k == 0), stop=False)
                    nc.tensor.matmul(psum[:, j, :], wk, t[:, j, :],
                                     start=False, stop=False)
                    nc.tensor.matmul(psum[:, j, :], wk, t2[:, j, :],
                                     start=False, stop=last)
                    if last:
                        # bank j is final: drain it while later banks finish
                        res = cp.tile([128, W], f32, tag=f"res{j}",
                                      name=f"res{j}")
                        nc.scalar.activation(res[:], psum[:, j, :], ACTF.Copy)
                        eng = nc.sync if j % 2 == 0 else nc.scalar
                        eng.dma_start(
                            out=out.rearrange("(j p) c -> p j c",
                                              p=128)[:, j],
                            in_=res[:])

    nc.compile()
    return nc


def kernel(input, weight, offset):
    global _compiled
    from concourse.bass_utils import run_bass_kernel_spmd

    if _compiled is None:
        _compiled = _build()
    nc = _compiled

    input = np.asarray(input, dtype=np.float32)
    offset = np.asarray(offset, dtype=np.float32)
    w9 = np.asarray(weight, dtype=np.float32).reshape(K * K)
    wdg = np.zeros((128, K * K, 128), np.float16)
    idx = np.arange(128)
    for k in range(K * K):
        wdg[idx, k, idx] = w9[k].astype(np.float16)

    ipad = np.zeros((B, HP, WP), np.float16)
    ipad[:, 1:H + 1, 1:W + 1] = input.astype(np.float16)

    # [B, 18, 512, 512] -> [B, p, k, e, j, c] fp16, contiguous per partition
    offp = np.ascontiguousarray(
        offset.reshape(B, K * K, 2, NBLK, 128, W).transpose(0, 4, 1, 2, 3, 5)
    ).astype(np.float16)

    in_maps = [
        {"ipad": ipad[b], "offp": offp[b], "wdg": wdg} for b in range(B)
    ]
    res = run_bass_kernel_spmd(nc, in_maps, list(range(NCORES)), trace=False)
    return np.stack([res.results[b]["out"] for b in range(B)], axis=0)


# revision 4
# speedup vs baseline: 1.2496x; 1.2496x over previous
import sys

for _p in ('/opt/trn_rl_repo', '/root/.axon_site'):
    if _p not in sys.path:
        sys.path.insert(0, _p)

import numpy as np

B, H, W = 8, 512, 512
K = 3
NCORES = 8
# padded image: 1 zero row/col before, 2 zero rows/cols after (cols padded
# further so shifted views stay in range and rows stay 4B-aligned)
HP, WP = H + 3, W + 8
NBLK = 4          # row blocks of 128 partitions packed along the free dim
AW = 520          # A tile width (Ipad cols 0..519)
DW = 516          # Dx/Dy/Dxy tile width

# GpSimd tensor_tensor measured: mutual serialization with DVE TT on the
# shared SBUF port pair (combined throughput BELOW DVE alone) — keep all
# elementwise on DVE.
GP_DX = ()
GP_DY = ()
GP_DXY = ()
GP_T_TAPS = ()

_compiled = None


def _build():
    import concourse.bacc as bacc
    import concourse.mybir as mybir
    from concourse.tile import TileContext

    f32, f16 = mybir.dt.float32, mybir.dt.float16
    ALU = mybir.AluOpType
    ACTF = mybir.ActivationFunctionType

    nc = bacc.Bacc("TRN2", target_bir_lowering=False, debug=False,
                   num_devices=NCORES)
    ipad = nc.dram_tensor("ipad", [HP, WP], f16, kind="ExternalInput")
    # offsets host-cast to fp16 and pre-packed to the SBUF tile layout:
    # offp[p, k, e, j, c] = offset[2k+e, 128j+p, c]  (e: 0=ly, 1=lx)
    offp = nc.dram_tensor("offp", [128, K * K, 2, NBLK, W], f16,
                          kind="ExternalInput")
    # stack of diag(w_k) matrices used as PE stationary weights
    wdg = nc.dram_tensor("wdg", [128, K * K, 128], f16, kind="ExternalInput")
    out = nc.dram_tensor("out", [H, W], f32, kind="ExternalOutput")

    with TileContext(nc) as tc:
        with (
            tc.tile_pool(name="img", bufs=1) as ip,
            tc.tile_pool(name="lylx", bufs=6) as lp,
            tc.tile_pool(name="tmp", bufs=3) as tp,
            tc.tile_pool(name="cst", bufs=1) as cp,
            tc.tile_pool(name="psum", bufs=1, space="PSUM") as pp,
        ):
            # image row-shifted copies (the two HWDGE rings), earliest first:
            # DVE prep for ky=-1 only needs A[-1], A[0]
            A = {}

            def load_img(dy, eng):
                A[dy] = ip.tile([128, NBLK, AW], f16, tag=f"A{dy}",
                                name=f"A{dy}")
                eng.dma_start(
                    out=A[dy][:],
                    in_=ipad[dy + 1:dy + 513, 0:AW].rearrange(
                        "(j p) c -> p j c", p=128))

            # per-tap (ly, lx) pair: one plain HWDGE load each, fp16 in HBM
            lylx = {}

            def load_lylx(k, eng):
                lylx[k] = lp.tile([128, 2, NBLK, W], f16, tag="l",
                                  name=f"l{k}")
                eng.dma_start(out=lylx[k][:], in_=offp[:, k])

            # order: the 2 images the ky=-1 prep needs, then the first two
            # offset pairs, then the rest — so DVE never waits mid-stream
            load_img(-1, nc.sync)
            load_img(0, nc.scalar)
            load_lylx(0, nc.sync)
            load_lylx(1, nc.scalar)
            wd = cp.tile([128, K * K, 128], f16, name="wd")
            nc.sync.dma_start(out=wd[:], in_=wdg[:])
            load_img(1, nc.scalar)
            load_img(2, nc.sync)
            for k in range(2, K * K):
                load_lylx(k, nc.sync if k % 2 == 0 else nc.scalar)

            psum = pp.tile([128, NBLK, W], f32, name="psum")

            Dx, Dy, Dxy = {}, {}, {}

            def make_dx(dy, eng):
                # Dx = horizontal difference of the padded image
                Dx[dy] = ip.tile([128, NBLK, DW], f16, tag=f"D{dy}",
                                 name=f"D{dy}")
                eng.tensor_tensor(Dx[dy][:], A[dy][:, :, 1:1 + DW],
                                  A[dy][:, :, 0:DW], ALU.subtract)

            def make_dy(j, eng):
                # Dy = vertical difference of the padded image
                Dy[j] = ip.tile([128, NBLK, DW], f16, tag=f"Y{j}",
                                name=f"Y{j}")
                eng.tensor_tensor(Dy[j][:], A[j + 1][:, :, 0:DW],
                                  A[j][:, :, 0:DW], ALU.subtract)

            def make_dxy(j, eng):
                # Dxy = vertical difference of Dx (cross term)
                Dxy[j] = ip.tile([128, NBLK, DW], f16, tag=f"X{j}",
                                 name=f"X{j}")
                eng.tensor_tensor(Dxy[j][:], Dx[j + 1][:],
                                  Dx[j][:], ALU.subtract)

            def eng_dx(dy):
                return nc.gpsimd if dy in GP_DX else nc.vector

            def eng_dy(j):
                return nc.gpsimd if j in GP_DY else nc.vector

            def eng_dxy(j):
                return nc.gpsimd if j in GP_DXY else nc.vector

            def iview(dy, q):
                return A[dy][:, :, q:q + W]

            # per tap: v*w_k = w_k*I0 + w_k*m0 + w_k*u
            #   m0 = lx*Dx[ky]
            #   u  = ly*(Dy[ky] + lx*Dxy[ky])
            for k in range(K * K):
                ky, kx = k // K - 1, k % K - 1
                q = kx + 1
                if kx == -1:
                    if ky not in Dx:
                        make_dx(ky, eng_dx(ky))
                    if ky + 1 not in Dx:
                        make_dx(ky + 1, eng_dx(ky + 1))
                    if ky not in Dy:
                        make_dy(ky, eng_dy(ky))
                    if ky not in Dxy:
                        make_dxy(ky, eng_dxy(ky))
                ll = lylx[k]
                ly, lx = ll[:, 0], ll[:, 1]

                t = tp.tile([128, NBLK, W], f16, tag="t", name="t")
                t2 = tp.tile([128, NBLK, W], f16, tag="t2", name="t2")
                t3 = tp.tile([128, NBLK, W], f16, tag="t3", name="t3")
                teng = nc.gpsimd if k in GP_T_TAPS else nc.vector
                teng.tensor_tensor(t[:], lx[:], Dx[ky][:, :, q:q + W],
                                   ALU.mult)
                nc.vector.tensor_tensor(t3[:], lx[:], Dxy[ky][:, :, q:q + W],
                                        ALU.mult)
                nc.vector.tensor_tensor(t2[:], t3[:], Dy[ky][:, :, q:q + W],
                                        ALU.add)
                nc.vector.tensor_tensor(t2[:], ly[:], t2[:], ALU.mult)

                wk = wd[:, k, :]
                last = k == K * K - 1
                for j in range(NBLK):
                    nc.tensor.matmul(psum[:, j, :], wk, iview(ky, q)[:, j, :],
                                     start=(k == 0), stop=False)
                    nc.tensor.matmul(psum[:, j, :], wk, t[:, j, :],
                                     start=False, stop=False)
                    nc.tensor.matmul(psum[:, j, :], wk, t2[:, j, :],
                                     start=False, stop=last)
                    if last:
                        # bank j is final: drain it while later banks finish
                        res = cp.tile([128, W], f32, tag=f"res{j}",
                                      name=f"res{j}")
                        nc.scalar.activation(res[:], psum[:, j, :], ACTF.Copy)
                        eng = nc.sync if j % 2 == 0 else nc.scalar
                        eng.dma_start(
                            out=out.rearrange("(j p) c -> p j c",
                                              p=128)[:, j],
                            in_=res[:])

    nc.compile()
    return nc


def kernel(input, weight, offset):
    global _compiled
    from concourse.bass_utils import run_bass_kernel_spmd

    if _compiled is None:
        _compiled = _build()
    nc = _compiled

    input = np.asarray(input, dtype=np.float32)
    offset = np.asarray(offset, dtype=np.float32)
    w9 = np.asarray(weight, dtype=np.float32).reshape(K * K)
    wdg = np.zeros((128, K * K, 128), np.float16)
    idx = np.arange(128)
    for k in range(K * K):
        wdg[idx, k, idx] = w9[k].astype(np.float16)

    ipad = np.zeros((B, HP, WP), np.float16)
    ipad[:, 1:H + 1, 1:W + 1] = input.astype(np.float16)

    # [B, 18, 512, 512] -> [B, p, k, e, j, c] fp16, contiguous per partition
    offp = np.ascontiguousarray(
        offset.reshape(B, K * K, 2, NBLK, 128, W).transpose(0, 4, 1, 2, 3, 5)
    ).astype(np.float16)

    in_maps = [
        {"ipad": ipad[b], "offp": offp[b], "wdg": wdg} for b in range(B)
    ]
    res = run_bass_kernel_spmd(nc, in_maps, list(range(NCORES)), trace=False)
    return np.stack([res.results[b]["out"] for b in range(B)], axis=0)
